# revision 7
# baseline (speedup 1.0000x reference)
"""Trainium2 Bass kernel for nn_ConstraintLayer (batched equality-constrained QP layer).

Math: the reference solves  M @ sol_i = [2*y_i; b_i]  for every batch row i,
with the SAME KKT matrix M = [[2I, A^T], [A, 0]] (80x80).  Since M is fixed,
    y_star = [2y, b] @ (M^{-1}[:64, :])^T  =  [y, b] @ Wc
with Wc = [Gy; Gb] (80x64), Gy = 2*Minv[:64,:64].T, Gb = Minv[:64,64:].T —
one skinny (batch,80)@(80,64) matmul, memory bound.

Distribution: pure data parallelism; the batch (1048576 rows) is split into 8
shards of 131072 rows, one per NeuronCore.  Wc is precomputed once on host
(float64 inverse) and replicated to every core.

Precision: the correctness gate is rel-err < 2e-2; a single fp16 pass
(fp16 inputs/weights, fp32 PSUM accumulate, fp16 output) measures ~7e-4 —
30x inside the gate — at 36 MB/core HBM traffic (20 MB in + 16 MB out)
against the ~358 GB/s per-core HBM roofline (~101 us).

Device layout (per core): the host packs each 512-row chunk feature-major as
an 80-partition moving tile ([64 y feats; 16 b feats] x 512 batch columns),
16 chunks per 2.5 MB block so every input DMA is a contiguous
[80-partition x 16KB] transfer and TensorE consumes the (80,512) tiles
directly — y and b arrive in ONE stream.

TensorE uses 128x64 COLUMN TILING (tile_position via PSUM base partition):
the K=80, M=64 stationary Wc is loaded into both column halves of the PE
array; even chunks stream through array cols 0-63 into PSUM partitions 0-63
while odd chunks stream through cols 64-127 into partitions 64-127
CONCURRENTLY — 2 moving columns/cycle, halving PE busy time vs a paired
K=128 blockdiag scheme, with no separate b matmul and a single 64-column
stationary reload per matmul.  Each PSUM bank [128,512] then holds two
finished chunks; PSUM->SBUF fp16 casts alternate between VectorE and
ScalarE (neither becomes the tail), and a contiguous 1MB fp16 DMA writes
the block out; the host inverts the packing.
"""

import numpy as np

BATCH = 1048576
IN_DIM = 64
OUT_DIM = 16
N_CORES = 8
SHARD = BATCH // N_CORES        # 131072
CHUNK = 512                     # batch rows per matmul (one PSUM half-bank col-span)
CH_PER_BLK = 32                 # chunks per input block: [80, 16384] f16 = 2.6 MB
N_BLK = SHARD // (CHUNK * CH_PER_BLK)   # 8
ICOLS = CHUNK * CH_PER_BLK      # 8192
OCOLS = ICOLS // 2              # 4096 (two chunks share a PSUM bank / out col-span)

_prog_cache = {}
last_results = None             # BassKernelResults of the most recent run (for test harness)


def _build_weights(A):
    """Host precompute of the stationary matrix (float64 inverse, fp16)."""
    m, n = A.shape  # (16, 64)
    A64 = np.asarray(A, dtype=np.float64)
    M = np.zeros((n + m, n + m))
    M[:n, :n] = 2.0 * np.eye(n)
    M[:n, n:] = A64.T
    M[n:, :n] = A64
    Minv = np.linalg.inv(M)
    Gy = (2.0 * Minv[:n, :n].T)          # (64, 64):  out = y @ Gy + b @ Gb
    Gb = (Minv[:n, n:].T)                # (16, 64)
    return np.concatenate([Gy, Gb], axis=0).astype(np.float16)   # Wc (80, 64)


def _pack_in(yh, bh):
    # (131072, 64)+(131072, 16) f16 -> blocks (16, 80, 8192);
    # partition = feature (0-63 y, 64-79 b), col = 512*chunk + s
    yv = yh.reshape(N_BLK, CH_PER_BLK, CHUNK, 64).transpose(0, 3, 1, 2)
    bv = bh.reshape(N_BLK, CH_PER_BLK, CHUNK, 16).transpose(0, 3, 1, 2)
    return np.ascontiguousarray(
        np.concatenate([yv.reshape(N_BLK, 64, ICOLS),
                        bv.reshape(N_BLK, 16, ICOLS)], axis=1))


def _unpack_out(ob):
    # (16, 128, 4096) f16 -> (131072, 64); partition = 64*(chunk%2) + feat,
    # col = 512*(chunk//2) + s  within each block of 16 chunks
    return np.ascontiguousarray(
        ob.reshape(N_BLK, 2, 64, CH_PER_BLK // 2, CHUNK).transpose(0, 3, 1, 4, 2)
    ).reshape(SHARD, 64)


def _build_program():
    import concourse.bacc as bacc
    import concourse.mybir as mybir
    import concourse.tile as tile

    f32 = mybir.dt.float32
    f16 = mybir.dt.float16
    nc = bacc.Bacc("TRN2")
    In_d = nc.dram_tensor("In", (N_BLK, 80, ICOLS), f16, kind="ExternalInput")
    Wc_d = nc.dram_tensor("Wc", (80, 64), f16, kind="ExternalInput")
    Ot = nc.dram_tensor("Ot", (N_BLK, 128, OCOLS), f16, kind="ExternalOutput")

    with tile.TileContext(nc) as tc:
        with (
            tc.tile_pool(name="wpool", bufs=1) as wpool,
            tc.tile_pool(name="ipool", bufs=3) as ipool,
            tc.tile_pool(name="opool", bufs=3) as opool,
            tc.tile_pool(name="pspool", bufs=8, space="PSUM") as pspool,
        ):
            wc = wpool.tile([80, 64], f16)
            nc.scalar.dma_start(wc[:], Wc_d[:])

            for blk in range(N_BLK):
                # alternate input blocks across the two HWDGE rings
                # (sync/scalar) for queue-depth; slice block 0 so the first
                # chunks land fast and the PE pipeline ramps early.
                itile = ipool.tile([80, ICOLS], f16, tag="in")
                in_ring = nc.sync if blk % 2 == 0 else nc.scalar
                if blk == 0:
                    q = ICOLS // 4
                    for k in range(4):
                        ring = nc.sync if k % 2 == 0 else nc.scalar
                        ring.dma_start(itile[:, k * q:(k + 1) * q],
                                       In_d[blk, :, k * q:(k + 1) * q])
                else:
                    in_ring.dma_start(itile[:], In_d[blk])
                otile = opool.tile([128, OCOLS], f16, tag="ot")
                for i in range(CH_PER_BLK // 2):
                    cols_e = slice((2 * i) * CHUNK, (2 * i + 1) * CHUNK)
                    cols_o = slice((2 * i + 1) * CHUNK, (2 * i + 2) * CHUNK)
                    ocols = slice(i * CHUNK, (i + 1) * CHUNK)
                    ps = pspool.tile([128, CHUNK], f32)
                    # 128x64 column tiling: same stationary in both column
                    # halves; the two chunk streams run CONCURRENTLY.
                    nc.tensor.matmul(ps[0:64, :], wc[:], itile[:, cols_e],
                                     start=True, stop=True)
                    nc.tensor.matmul(ps[64:128, :], wc[:], itile[:, cols_o],
                                     start=True, stop=True)
                    # split PSUM->SBUF casts across the two free compute
                    # engines so neither becomes the dependency tail
                    if i % 2 == 0:
                        nc.vector.tensor_copy(otile[:, ocols], ps[:])
                    else:
                        nc.scalar.copy(otile[:, ocols], ps[:])
                    # flush finished half-blocks so the output stream
                    # overlaps instead of bursting at block end
                    if i == CH_PER_BLK // 4 - 1:
                        nc.gpsimd.dma_start(Ot[blk, :, :OCOLS // 2],
                                            otile[:, :OCOLS // 2])
                nc.gpsimd.dma_start(Ot[blk, :, OCOLS // 2:],
                                    otile[:, OCOLS // 2:])
    nc.compile()  # bacc passes: split sync waits to HW limits, alloc regs, DCE
    return nc


def _get_program():
    if "nc" not in _prog_cache:
        _prog_cache["nc"] = _build_program()
    return _prog_cache["nc"]


def kernel(y, A, b):
    global last_results
    from concourse.bass_utils import run_bass_kernel_spmd

    y = np.ascontiguousarray(np.asarray(y, dtype=np.float32))
    b = np.ascontiguousarray(np.asarray(b, dtype=np.float32))
    A = np.asarray(A, dtype=np.float32)
    assert y.shape == (BATCH, IN_DIM) and b.shape == (BATCH, OUT_DIM)

    Wc = _build_weights(A)
    yh = y.astype(np.float16)
    bh = b.astype(np.float16)

    in_maps = []
    for core in range(N_CORES):
        sl = slice(core * SHARD, (core + 1) * SHARD)
        in_maps.append({"In": _pack_in(yh[sl], bh[sl]), "Wc": Wc})

    nc = _get_program()
    res = run_bass_kernel_spmd(nc, in_maps, core_ids=list(range(N_CORES)))
    last_results = res

    out = np.empty((BATCH, IN_DIM), np.float32)
    for core in range(N_CORES):
        out[core * SHARD:(core + 1) * SHARD] = _unpack_out(res.results[core]["Ot"])
    return out


# revision 9
# speedup vs baseline: 1.0818x; 1.0818x over previous
"""Trainium2 Bass kernel for nn_ConstraintLayer (batched equality-constrained QP layer).

Math: the reference solves  M @ sol_i = [2*y_i; b_i]  for every batch row i,
with the SAME KKT matrix M = [[2I, A^T], [A, 0]] (80x80).  Since M is fixed,
    y_star = [2y, b] @ (M^{-1}[:64, :])^T  =  [y, b] @ Wc
with Wc = [Gy; Gb] (80x64), Gy = 2*Minv[:64,:64].T, Gb = Minv[:64,64:].T —
one skinny (batch,80)@(80,64) matmul, memory bound.

Distribution: pure data parallelism; the batch (1048576 rows) is split into 8
shards of 131072 rows, one per NeuronCore.  Wc is precomputed once on host
(float64 inverse) and replicated to every core.

Precision: the correctness gate is rel-err < 2e-2; a single fp16 pass
(fp16 inputs/weights, fp32 PSUM accumulate, fp16 output) measures ~7e-4 —
30x inside the gate — at 36 MB/core HBM traffic (20 MB in + 16 MB out)
against the ~358 GB/s per-core HBM roofline (~101 us).

Device layout (per core): the host packs each 512-row chunk feature-major as
an 80-partition moving tile ([64 y feats; 16 b feats] x 512 batch columns),
16 chunks per 2.5 MB block so every input DMA is a contiguous
[80-partition x 16KB] transfer and TensorE consumes the (80,512) tiles
directly — y and b arrive in ONE stream.

TensorE uses 128x64 COLUMN TILING (tile_position via PSUM base partition):
the K=80, M=64 stationary Wc is loaded into both column halves of the PE
array; even chunks stream through array cols 0-63 into PSUM partitions 0-63
while odd chunks stream through cols 64-127 into partitions 64-127
CONCURRENTLY — 2 moving columns/cycle, halving PE busy time vs a paired
K=128 blockdiag scheme, with no separate b matmul and a single 64-column
stationary reload per matmul.  Each PSUM bank [128,512] then holds two
finished chunks; PSUM->SBUF fp16 casts alternate between VectorE and
ScalarE (neither becomes the tail), and a contiguous 1MB fp16 DMA writes
the block out; the host inverts the packing.
"""

import numpy as np

BATCH = 1048576
IN_DIM = 64
OUT_DIM = 16
N_CORES = 8
SHARD = BATCH // N_CORES        # 131072
CHUNK = 512                     # batch rows per matmul (one PSUM half-bank col-span)
CH_PER_BLK = 32                 # chunks per input block: [80, 16384] f16 = 2.6 MB
N_BLK = SHARD // (CHUNK * CH_PER_BLK)   # 8
ICOLS = CHUNK * CH_PER_BLK      # 8192
OCOLS = ICOLS // 2              # 4096 (two chunks share a PSUM bank / out col-span)

_prog_cache = {}
last_results = None             # BassKernelResults of the most recent run (for test harness)


def _build_weights(A):
    """Host precompute of the stationary matrix (float64 inverse, fp16)."""
    m, n = A.shape  # (16, 64)
    A64 = np.asarray(A, dtype=np.float64)
    M = np.zeros((n + m, n + m))
    M[:n, :n] = 2.0 * np.eye(n)
    M[:n, n:] = A64.T
    M[n:, :n] = A64
    Minv = np.linalg.inv(M)
    Gy = (2.0 * Minv[:n, :n].T)          # (64, 64):  out = y @ Gy + b @ Gb
    Gb = (Minv[:n, n:].T)                # (16, 64)
    return np.concatenate([Gy, Gb], axis=0).astype(np.float16)   # Wc (80, 64)


def _pack_in(yh, bh):
    # (131072, 64)+(131072, 16) f16 -> blocks (16, 80, 8192);
    # partition = feature (0-63 y, 64-79 b), col = 512*chunk + s
    yv = yh.reshape(N_BLK, CH_PER_BLK, CHUNK, 64).transpose(0, 3, 1, 2)
    bv = bh.reshape(N_BLK, CH_PER_BLK, CHUNK, 16).transpose(0, 3, 1, 2)
    return np.ascontiguousarray(
        np.concatenate([yv.reshape(N_BLK, 64, ICOLS),
                        bv.reshape(N_BLK, 16, ICOLS)], axis=1))


def _unpack_out(ob):
    # (16, 128, 4096) f16 -> (131072, 64); partition = 64*(chunk%2) + feat,
    # col = 512*(chunk//2) + s  within each block of 16 chunks
    return np.ascontiguousarray(
        ob.reshape(N_BLK, 2, 64, CH_PER_BLK // 2, CHUNK).transpose(0, 3, 1, 4, 2)
    ).reshape(SHARD, 64)


def _build_program():
    import concourse.bacc as bacc
    import concourse.mybir as mybir
    import concourse.tile as tile

    f32 = mybir.dt.float32
    f16 = mybir.dt.float16
    nc = bacc.Bacc("TRN2")
    In_d = nc.dram_tensor("In", (N_BLK, 80, ICOLS), f16, kind="ExternalInput")
    Wc_d = nc.dram_tensor("Wc", (80, 64), f16, kind="ExternalInput")
    Ot = nc.dram_tensor("Ot", (N_BLK, 128, OCOLS), f16, kind="ExternalOutput")

    with tile.TileContext(nc) as tc:
        with (
            tc.tile_pool(name="wpool", bufs=1) as wpool,
            tc.tile_pool(name="ipool", bufs=3) as ipool,
            tc.tile_pool(name="opool", bufs=3) as opool,
            tc.tile_pool(name="pspool", bufs=8, space="PSUM") as pspool,
        ):
            wc = wpool.tile([80, 64], f16)
            nc.scalar.dma_start(wc[:], Wc_d[:])

            for blk in range(N_BLK):
                # alternate input blocks across the two HWDGE rings
                # (sync/scalar) for queue-depth; slice block 0 so the first
                # chunks land fast and the PE pipeline ramps early.
                itile = ipool.tile([80, ICOLS], f16, tag="in")
                in_ring = nc.sync if blk % 2 == 0 else nc.scalar
                in_ring.dma_start(itile[:], In_d[blk])
                otile = opool.tile([128, OCOLS], f16, tag="ot")
                for i in range(CH_PER_BLK // 2):
                    cols_e = slice((2 * i) * CHUNK, (2 * i + 1) * CHUNK)
                    cols_o = slice((2 * i + 1) * CHUNK, (2 * i + 2) * CHUNK)
                    ocols = slice(i * CHUNK, (i + 1) * CHUNK)
                    ps = pspool.tile([128, CHUNK], f32)
                    # 128x64 column tiling: same stationary in both column
                    # halves; the two chunk streams run CONCURRENTLY.
                    nc.tensor.matmul(ps[0:64, :], wc[:], itile[:, cols_e],
                                     start=True, stop=True)
                    nc.tensor.matmul(ps[64:128, :], wc[:], itile[:, cols_o],
                                     start=True, stop=True)
                    # split PSUM->SBUF casts across the two free compute
                    # engines so neither becomes the dependency tail
                    if i % 2 == 0:
                        nc.vector.tensor_copy(otile[:, ocols], ps[:])
                    else:
                        nc.scalar.copy(otile[:, ocols], ps[:])
                nc.gpsimd.dma_start(Ot[blk], otile[:])
    nc.compile()  # bacc passes: split sync waits to HW limits, alloc regs, DCE
    return nc


def _get_program():
    if "nc" not in _prog_cache:
        _prog_cache["nc"] = _build_program()
    return _prog_cache["nc"]


def kernel(y, A, b):
    global last_results
    from concourse.bass_utils import run_bass_kernel_spmd

    y = np.ascontiguousarray(np.asarray(y, dtype=np.float32))
    b = np.ascontiguousarray(np.asarray(b, dtype=np.float32))
    A = np.asarray(A, dtype=np.float32)
    assert y.shape == (BATCH, IN_DIM) and b.shape == (BATCH, OUT_DIM)

    Wc = _build_weights(A)
    yh = y.astype(np.float16)
    bh = b.astype(np.float16)

    in_maps = []
    for core in range(N_CORES):
        sl = slice(core * SHARD, (core + 1) * SHARD)
        in_maps.append({"In": _pack_in(yh[sl], bh[sl]), "Wc": Wc})

    nc = _get_program()
    res = run_bass_kernel_spmd(nc, in_maps, core_ids=list(range(N_CORES)))
    last_results = res

    out = np.empty((BATCH, IN_DIM), np.float32)
    for core in range(N_CORES):
        out[core * SHARD:(core + 1) * SHARD] = _unpack_out(res.results[core]["Ot"])
    return out
